# revision 55
# baseline (speedup 1.0000x reference)
"""DeepSet (segment_reduce) Trainium2 kernel — v2.

Model (per reference):
    h  = relu(relu(x @ w1 + b1) @ w2 + b2)          # phi, per track
    pooled[e] = sum_{t in event e} h[t]             # segment sum (sorted ids)
    y  = sigmoid(relu(relu(pooled@rw1+rb1)@rw2+rb2)@rw3+rb3)   # rho, per event

Strategy (8 NeuronCores, SPMD single program):
  - Shard tracks in fixed 250k blocks per core (NOT event aligned); the few
    boundary events that straddle cores are recomputed exactly on the host
    (tiny) and patched into the output.
  - Host reorders x into a transposed interleaved layout xt4 (bf16) so the
    device streams it with contiguous DMA straight into the PE.
  - phi mm1: one [64,128]-stationary matmul per (hab, h) with row tiling so
    the two hab halves run concurrently on the PE; ACT applies bias+relu.
  - phi mm2: h1 chunks as stationary (bf16 128-col -> FWL), w2stk moving.
  - pooling (mm3): FLIPPED vs v1 — stationary = h2 chunk [128 trk, 64 lat],
    moving = a host-precomputed onehot [128 trk, 64 slot-cols] (fp8, DMA'd),
    output = pooledT [64 lat, slot-cols] accumulated into PSUM banks that
    hold 512 slot-columns each.  Slots live on PSUM *columns*, so windows
    need no 32-partition alignment; each bank is cleared up front (memset)
    and every matmul uses start=False (DVE writes don't set has_written, so
    the first matmul overwrites the zeros and the rest accumulate).  Only
    windows that cross a 512-col bank boundary (w % 16 == 15) split into
    two matmuls.
  - rho: operates directly on pooledT slices (no PE transposes), bf16.
  - Boundary events / event ids that never appear are patched on host.
"""

import numpy as np
import ml_dtypes
from contextlib import ExitStack

import concourse.bass as bass
import concourse.tile as tile
from concourse import bacc, mybir
from concourse.bass_utils import run_bass_kernel_spmd

BF16 = ml_dtypes.bfloat16
FP8 = ml_dtypes.float8_e4m3
FP32 = np.float32
AF = mybir.ActivationFunctionType
ALU = mybir.AluOpType
dt = mybir.dt


class Cfg:
    def __init__(self, n_cores=8, tracks_per_core=250_000, tiles_per_window=4,
                 oh_dtype="float8e4", clear_engine="vector",
                 h2_act_every=0, flush_engine="scalar", rho_dtype="float16"):
        self.n_cores = n_cores
        self.F = 32           # input features
        self.L = 64           # latent width (phi hidden and output width)
        self.RH = 128         # rho hidden width
        self.T_core = tracks_per_core
        self.G = 4096         # tracks per DMA super-tile
        self.TPAD = ((tracks_per_core + 128 + self.G - 1) // self.G) * self.G
        self.NT = self.TPAD // 128          # 128-track tiles per core
        self.TPW = tiles_per_window         # tiles per 32-slot window
        self.NWIN = self.NT // self.TPW     # windows (tile i -> window i//TPW)
        # slot columns: window w covers cols [32w, 32w+64)
        self.SPAD = ((32 * (self.NWIN - 1) + 64 + 511) // 512) * 512
        self.NBANK = self.SPAD // 512       # 512-slot psum banks
        self.oh_dtype = oh_dtype
        self.clear_engine = clear_engine
        self.h2_act_every = h2_act_every    # every k-th h2 relu on ACT (0=never)
        self.flush_engine = flush_engine
        self.rho_dtype = rho_dtype

    def __repr__(self):
        return (f"Cfg2(cores={self.n_cores},TPAD={self.TPAD},NT={self.NT},"
                f"TPW={self.TPW},SPAD={self.SPAD},oh={self.oh_dtype},"
                f"clr={self.clear_engine},h2a={self.h2_act_every},"
                f"fl={self.flush_engine},rho={self.rho_dtype})")


FULL_CFG = Cfg()


# --------------------------------------------------------------------------
# Host-side planning
# --------------------------------------------------------------------------

class ScheduleOverflow(Exception):
    pass


def compact_ranks(event_ids):
    ev = np.asarray(event_ids)
    change = (ev[1:] != ev[:-1]).astype(np.int64)
    r = np.concatenate([[0], np.cumsum(change)]).astype(np.int64)
    return r


def plan_core(r_local, cfg):
    """Assign slot columns to local events.

    r_local: int64 [Tc] local event ranks (0-based, non-decreasing).
    Returns (rel int16 [TPAD] per-track window-relative column (or -1),
             slot int64 [n_local_events] global slot column per event).
    """
    Tc = len(r_local)
    n_ev = int(r_local[-1]) + 1 if Tc else 0
    first_track = np.searchsorted(r_local, np.arange(n_ev), side="left")
    last_track = np.searchsorted(r_local, np.arange(n_ev), side="right") - 1
    first_w = (first_track // 128) // cfg.TPW
    last_w = (last_track // 128) // cfg.TPW

    slot = np.zeros(n_ev, dtype=np.int64)
    counter = 0
    lo = 32 * last_w
    for e in range(n_ev):
        counter = max(counter, lo[e])
        slot[e] = counter
        counter += 1
    rel_hi = slot - 32 * first_w
    if rel_hi.max(initial=0) >= 64:
        raise ScheduleOverflow(f"max rel {rel_hi.max()} >= 64")
    if slot.max(initial=0) >= cfg.SPAD:
        raise ScheduleOverflow("slot overflow")

    tiles = np.arange(cfg.TPAD) // 128
    rel = np.full(cfg.TPAD, -1, dtype=np.int64)
    rel[:Tc] = slot[r_local] - 32 * (tiles[:Tc] // cfg.TPW)
    assert rel[:Tc].min(initial=0) >= 0 and rel[:Tc].max(initial=0) < 64
    return rel, slot


def make_xt4(x_pad, cfg):
    """[TPAD, F] f32 -> [128, TPAD//4] interleaved transposed layout.

    track t = 4096 g + 1024 b + j maps to partition 32 b + f, column
    1024 g + j.
    """
    G = cfg.G
    ng = cfg.TPAD // G
    xt = x_pad.reshape(ng, 4, G // 4, cfg.F).transpose(1, 3, 0, 2)
    return np.ascontiguousarray(xt.reshape(128, -1))


def emission_order(cfg):
    """Tile indices in device processing order (must match build_program)."""
    order = []
    for g in range(cfg.TPAD // cfg.G):
        for hab in range(2):
            i0 = 32 * g + 16 * hab
            for t2 in range(2):
                for m in range(8):
                    order.append(i0 + 8 * (m % 2) + 4 * t2 + m // 2)
    return order


def make_onehot(rel, cfg):
    """[TPAD] window-relative cols -> [128, NT*64] onehot in emission order."""
    order = emission_order(cfg)
    rel_t = rel.reshape(cfg.NT, 128)[order]          # [NT(pos), 128]
    oh = rel_t[:, :, None] == np.arange(64)[None, None, :]
    np_dt = FP8 if cfg.oh_dtype == "float8e4" else BF16
    oh = oh.transpose(1, 0, 2).reshape(128, cfg.NT * 64)
    return np.ascontiguousarray(oh.astype(np_dt))


def phi_numpy(x, w1, b1, w2, b2, *unused):
    h = np.maximum(x @ w1 + b1, 0.0)
    h = np.maximum(h @ w2 + b2, 0.0)
    return h


def rho_numpy(pooled, rw1, rb1, rw2, rb2, rw3, rb3):
    r = np.maximum(pooled @ rw1 + rb1, 0.0)
    r = np.maximum(r @ rw2 + rb2, 0.0)
    z = r @ rw3 + rb3
    return 1.0 / (1.0 + np.exp(-z))


# --------------------------------------------------------------------------
# Device program
# --------------------------------------------------------------------------

def build_program(cfg):
    nc = bacc.Bacc("TRN2", target_bir_lowering=False, debug=False,
                   enable_asserts=False, num_devices=cfg.n_cores)
    F, L, RH = cfg.F, cfg.L, cfg.RH
    NT, TPW = cfg.NT, cfg.TPW
    ohdt = getattr(dt, cfg.oh_dtype)
    rhodt = getattr(dt, cfg.rho_dtype)

    xt4_d = nc.dram_tensor("xt4", [128, cfg.TPAD // 4], dt.bfloat16,
                           kind="ExternalInput").ap()
    oh_d = nc.dram_tensor("ohd", [128, NT * 64], ohdt,
                          kind="ExternalInput").ap()
    w1_d = nc.dram_tensor("w1blk", [128, 256], dt.bfloat16,
                          kind="ExternalInput").ap()
    b1_d = nc.dram_tensor("b1rep", [128, 1], dt.float32,
                          kind="ExternalInput").ap()
    w2_d = nc.dram_tensor("w2stk", [128, 128], dt.bfloat16,
                          kind="ExternalInput").ap()
    rw1_d = nc.dram_tensor("rw1", [64, RH], rhodt,
                           kind="ExternalInput").ap()
    rb1_d = nc.dram_tensor("rb1", [128, 1], dt.float32,
                           kind="ExternalInput").ap()
    rw2_d = nc.dram_tensor("rw2", [128, L], rhodt,
                           kind="ExternalInput").ap()
    rb2_d = nc.dram_tensor("rb2", [64, 1], dt.float32,
                           kind="ExternalInput").ap()
    rw3_d = nc.dram_tensor("rw3", [64, 1], rhodt,
                           kind="ExternalInput").ap()
    y_d = nc.dram_tensor("y", [1, cfg.SPAD], dt.float32,
                         kind="ExternalOutput").ap()

    NG = cfg.TPAD // cfg.G

    with tile.TileContext(nc) as tc, ExitStack() as ctx:
        const = ctx.enter_context(tc.tile_pool(name="const", bufs=1))
        w1_s = const.tile([128, 256], dt.bfloat16, tag="w1")
        b1_s = const.tile([128, 1], dt.float32, tag="b1")
        w2_s = const.tile([128, 128], dt.bfloat16, tag="w2")

        pooled_pool = ctx.enter_context(tc.tile_pool(name="pooled", bufs=1))
        pooled = pooled_pool.tile([64, cfg.SPAD], rhodt)

        clear_eng = getattr(nc, cfg.clear_engine)
        flush_eng = getattr(nc, cfg.flush_engine)

        # rho constants allocated up front: the rho tail is interleaved into
        # the main loop at bank-flush points (DMAs issued after the g=0
        # prefetch below)
        rho_const = ctx.enter_context(tc.tile_pool(name="rhoc", bufs=1))
        rw1_s = rho_const.tile([64, RH], rhodt, tag="rw1")
        rb1_s = rho_const.tile([128, 1], dt.float32, tag="rb1")
        rw2_s = rho_const.tile([128, L], rhodt, tag="rw2")
        rb2_s = rho_const.tile([64, 1], dt.float32, tag="rb2")
        rw3_s = rho_const.tile([64, 1], rhodt, tag="rw3")

        # ---------------- main loop ----------------
        bank_tiles = {}     # bank index -> psum tile object
        with (
            tc.tile_pool(name="xt", bufs=8) as xt_pool,
            tc.tile_pool(name="ohp", bufs=8) as oh_pool,
            tc.tile_pool(name="p1", bufs=2, space="PSUM") as p1_pool,
            tc.tile_pool(name="h1", bufs=8) as h1_pool,
            tc.tile_pool(name="p2", bufs=2, space="PSUM") as p2_pool,
            tc.tile_pool(name="h2", bufs=8) as h2_pool,
            tc.tile_pool(name="p3", bufs=2, space="PSUM") as p3_pool,
            tc.tile_pool(name="r1s", bufs=3) as r1s_pool,
            tc.tile_pool(name="r2s", bufs=3) as r2s_pool,
            tc.tile_pool(name="ys", bufs=2) as ys_pool,
        ):
            def get_bank(B):
                if B not in bank_tiles:
                    bt = p3_pool.tile([64, 512], dt.float32, tag="bank",
                                      name=f"bank{B}")
                    clear_eng.memset(bt[:], 0.0)
                    bank_tiles[B] = bt
                return bank_tiles[B]

            def flush_bank(B):
                bt = bank_tiles.pop(B)
                if cfg.flush_engine == "scalar":
                    nc.scalar.activation(pooled[:, 512 * B:512 * (B + 1)],
                                         bt[:], AF.Copy)
                else:
                    flush_eng.tensor_copy(pooled[:, 512 * B:512 * (B + 1)],
                                          bt[:])

            def mm3(i, h2_ap, oh_ap):
                w = i // TPW
                c = 32 * w
                B, off = c // 512, c % 512
                if off <= 448:
                    nc.tensor.matmul(get_bank(B)[:, off:off + 64],
                                     h2_ap, oh_ap,
                                     start=False, stop=True,
                                     skip_group_check=True)
                else:  # window straddles two banks: split 32 + 32
                    nc.tensor.matmul(get_bank(B)[:, off:off + 32],
                                     h2_ap, oh_ap[:, 0:32],
                                     start=False, stop=True,
                                     skip_group_check=True)
                    nc.tensor.matmul(get_bank(B + 1)[:, 0:32],
                                     h2_ap, oh_ap[:, 32:64],
                                     start=False, stop=True,
                                     skip_group_check=True)

            # rho tail stages, interleaved at bank-flush points; PSUM for the
            # small tail matmuls is borrowed from the p2 pool (all tail tiles
            # are consumed within the same super-tile)
            stages = {}

            def tail_a(B):
                r1p = p2_pool.tile([128, 512], dt.float32, tag="p2",
                                   name="r1p")
                nc.tensor.matmul(r1p[:], rw1_s[:],
                                 pooled[:, 512 * B:512 * (B + 1)],
                                 start=True, stop=True)
                r1s = r1s_pool.tile([128, 512], rhodt, tag="r1s")
                nc.scalar.activation(r1s[:], r1p[:], AF.Relu, bias=rb1_s[:])
                stages[("b", B)] = r1s

            def tail_b(B):
                r1s = stages.pop(("b", B))
                r2p = p2_pool.tile([128, 512], dt.float32, tag="p2",
                                   name="r2p")
                nc.tensor.matmul(r2p[0:64, :], rw2_s[:], r1s[:],
                                 start=True, stop=True)
                r2s = r2s_pool.tile([64, 512], rhodt, tag="r2s")
                # rho_b2 == 0 (asserted host-side); the tail has two
                # super-tiles of slack so DVE latency is immaterial
                nc.vector.tensor_scalar_max(r2s[:], r2p[0:64, :], 0.0)
                stages[("c", B)] = r2s

            def tail_c(B):
                r2s = stages.pop(("c", B))
                yp = p2_pool.tile([128, 512], dt.float32, tag="p2",
                                  name="yp")
                nc.tensor.matmul(yp[0:1, :], rw3_s[:], r2s[:],
                                 start=True, stop=True)
                ys = ys_pool.tile([1, 512], dt.float32, tag="ys")
                nc.vector.tensor_copy(ys[:], yp[0:1, :])
                nc.sync.dma_start(y_d[:, 512 * B:512 * (B + 1)], ys[:])

            # first super-tile and w1 go down the DMA queue ahead of the
            # other constants so mm1 of g=0 starts as early as possible
            xt0 = xt_pool.tile([128, 1024], dt.bfloat16, tag="xt", name="xt0")
            nc.sync.dma_start(xt0[:], xt4_d[:, 0:1024])
            nc.sync.dma_start(w1_s[:], w1_d)
            oh0 = oh_pool.tile([128, 2048], ohdt, tag="oh", name="oh0")
            nc.sync.dma_start(oh0[:], oh_d[:, 0:2048])
            nc.sync.dma_start(b1_s[:], b1_d)
            nc.sync.dma_start(w2_s[:], w2_d)
            nc.sync.dma_start(rw1_s[:], rw1_d)
            nc.sync.dma_start(rb1_s[:], rb1_d)
            nc.sync.dma_start(rw2_s[:], rw2_d)
            nc.sync.dma_start(rb2_s[:], rb2_d)
            nc.sync.dma_start(rw3_s[:], rw3_d)

            t2_count = 0
            for g in range(NG):
                if g == 0:
                    xt_t, oh_t = xt0, oh0
                else:
                    xt_t = xt_pool.tile([128, 1024], dt.bfloat16, tag="xt")
                    nc.sync.dma_start(xt_t[:],
                                      xt4_d[:, 1024 * g:1024 * (g + 1)])
                    oh_t = oh_pool.tile([128, 2048], ohdt, tag="oh")
                    nc.sync.dma_start(oh_t[:],
                                      oh_d[:, 2048 * g:2048 * (g + 1)])
                p1s = [p1_pool.tile([128, 1024], dt.float32, tag="p1",
                                    name=f"p1_{hab}") for hab in range(2)]
                # full-row stationaries (zero half per hab): untiled LDWEIGHTS
                # can load into the background buffer and hide behind matmuls
                for h in range(2):
                    for hab in range(2):
                        nc.tensor.matmul(
                            p1s[hab][:, 512 * h:512 * (h + 1)],
                            w1_s[:, 128 * hab:128 * (hab + 1)],
                            xt_t[:, 512 * h:512 * (h + 1)],
                            start=True, stop=True)
                h1s = []
                for hab in range(2):
                    h1 = h1_pool.tile([128, 1024], dt.bfloat16, tag="h1",
                                      name=f"h1_{hab}")
                    nc.scalar.activation(h1[:], p1s[hab][:],
                                         AF.Relu, bias=b1_s[:])
                    h1s.append(h1)
                if g % 2 == 0 and g >= 2:
                    # oldest stage first: every tail matmul's input is >=1
                    # super-tile old, so the PE never waits here
                    A = (g - 2) // 2
                    if A >= 2:
                        tail_c(A - 2)
                    if A >= 1:
                        tail_b(A - 1)
                    tail_a(A)
                for hab in range(2):
                    h1 = h1s[hab]
                    i0 = 32 * g + 16 * hab
                    for t2 in range(2):
                        p2 = p2_pool.tile([128, 512], dt.float32, tag="p2")
                        for m4 in range(4):
                            j = 4 * t2 + m4
                            nc.tensor.matmul(
                                p2[:, 128 * m4:128 * (m4 + 1)],
                                h1[:, 128 * j:128 * (j + 1)],
                                w2_s[:],
                                start=True, stop=True)
                        h2 = h2_pool.tile([128, 512], dt.bfloat16, tag="h2")
                        t2_count += 1
                        if (cfg.h2_act_every and
                                t2_count % cfg.h2_act_every == 0):
                            nc.scalar.activation(h2[:], p2[:], AF.Relu)
                        else:
                            nc.vector.tensor_scalar_max(h2[:], p2[:], 0.0)
                        ohbase = 64 * (16 * hab + 8 * t2)
                        for m in range(8):
                            i = i0 + 8 * (m % 2) + 4 * t2 + m // 2
                            mm3(i, h2[:, 64 * m:64 * (m + 1)],
                                oh_t[:, ohbase + 64 * m:ohbase + 64 * (m + 1)])
                if g % 2 == 1:
                    B = (g - 1) // 2
                    if B in bank_tiles:
                        flush_bank(B)
            for B in sorted(bank_tiles):
                flush_bank(B)
            # drain the remaining tail stages
            NBK = cfg.NBANK
            last_even = NG - 1 - ((NG - 1) % 2)
            a_next = (last_even - 2) // 2 + 1 if last_even >= 2 else 0
            for A in range(a_next, NBK + 3):
                if 0 <= A - 2 < NBK:
                    tail_c(A - 2)
                if 1 <= A - 1 < NBK:
                    tail_b(A - 1)
                if A < NBK:
                    tail_a(A)

    nc.compile()
    return nc


# --------------------------------------------------------------------------
# kernel() entry point
# --------------------------------------------------------------------------

_PROG_CACHE = {}
TRACE = False
_LAST_RES = None


def _install_ntff_hook():
    """Register the axon NTFF profiling hook if the image lacks
    antenv.axon_hooks (needed for run_bass_kernel_spmd(trace=True))."""
    import sys, types
    try:
        from antenv.axon_hooks import get_axon_ntff_profile_hook  # noqa: F401
        return True
    except ImportError:
        pass
    try:
        from trn_agent_boot.trn_boot import _ntff_profile_via_ctypes
        hook = _ntff_profile_via_ctypes("/opt/axon/libaxon_pjrt.so")
        if hook is None:
            return False
        mod = types.ModuleType("antenv.axon_hooks")
        mod.get_axon_ntff_profile_hook = lambda: hook
        mod.set_axon_ntff_profile_hook = lambda h: None
        sys.modules["antenv.axon_hooks"] = mod
        return True
    except Exception:
        return False


def _get_program(cfg):
    key = repr(cfg)
    if key not in _PROG_CACHE:
        _PROG_CACHE[key] = build_program(cfg)
    return _PROG_CACHE[key]


def prepare_in_maps(inputs, cfg):
    x = np.asarray(inputs["x"], np.float32)
    ev = np.asarray(inputs["event_ids"])
    w1 = np.asarray(inputs["phi_w1"], np.float32)
    b1 = np.asarray(inputs["phi_b1"], np.float32)
    w2 = np.asarray(inputs["phi_w2"], np.float32)
    b2 = np.asarray(inputs["phi_b2"], np.float32)
    assert np.all(b2 == 0.0), "phi_b2 != 0 unsupported fast path"
    assert np.all(np.asarray(inputs["rho_b2"]) == 0.0), \
        "rho_b2 != 0 unsupported fast path"
    T = x.shape[0]
    r = compact_ranks(ev)
    D = int(r[-1]) + 1

    rho_np = {"bfloat16": BF16, "float16": np.float16}.get(
        cfg.rho_dtype, np.float32)

    blk = np.zeros((64, 128), np.float32)
    blk[0:32, 0:64] = w1
    blk[32:64, 64:128] = w1
    z = np.zeros((64, 128), np.float32)
    w1blk = np.hstack([np.vstack([blk, z]),
                       np.vstack([z, blk])]).astype(BF16)
    w2stk = np.zeros((128, 128), np.float32)
    w2stk[0:64, 0:64] = w2
    w2stk[64:128, 64:128] = w2
    w2stk = w2stk.astype(BF16)
    b1rep = np.tile(b1.reshape(-1), 2).reshape(128, 1).astype(np.float32)
    rw1 = np.asarray(inputs["rho_w1"], np.float32).astype(rho_np)
    rb1 = np.asarray(inputs["rho_b1"], np.float32).reshape(128, 1)
    rw2 = np.asarray(inputs["rho_w2"], np.float32).astype(rho_np)
    rb2 = np.asarray(inputs["rho_b2"], np.float32).reshape(64, 1)
    rw3 = np.asarray(inputs["rho_w3"], np.float32).astype(rho_np)

    in_maps, metas = [], []
    for c in range(cfg.n_cores):
        s, e = c * cfg.T_core, min((c + 1) * cfg.T_core, T)
        r_loc_g = r[s:e]
        e0 = int(r_loc_g[0])
        r_loc = (r_loc_g - e0).astype(np.int64)
        rel, slot = plan_core(r_loc, cfg)
        xp = np.zeros((cfg.TPAD, cfg.F), np.float32)
        xp[:e - s] = x[s:e]
        in_maps.append({
            "xt4": make_xt4(xp, cfg).astype(BF16),
            "ohd": make_onehot(rel, cfg),
            "w1blk": w1blk, "b1rep": b1rep, "w2stk": w2stk,
            "rw1": rw1, "rb1": rb1, "rw2": rw2, "rb2": rb2, "rw3": rw3,
        })
        # events fully owned by this core (not straddling boundary)
        n_ev = int(r_loc[-1]) + 1
        own_lo = 0 if s == 0 else (1 if r[s - 1] == r[s] else 0)
        own_hi = n_ev if e == T else (n_ev - 1 if r[e - 1] == r[e] else n_ev)
        metas.append(dict(e0=e0, n_ev=n_ev, own_lo=own_lo, own_hi=own_hi,
                          slot=slot))
    return in_maps, metas, r, D


def assemble_output(results, metas, r, D, inputs, cfg, n_events):
    x = np.asarray(inputs["x"], np.float32)
    args = [np.asarray(inputs[k], np.float32) for k in
            ("phi_w1", "phi_b1", "phi_w2", "phi_b2")]
    rargs = [np.asarray(inputs[k], np.float32) for k in
             ("rho_w1", "rho_b1", "rho_w2", "rho_b2", "rho_w3", "rho_b3")]
    y = np.empty(n_events, np.float32)
    if D < n_events:
        y[D:] = rho_numpy(np.zeros((1, cfg.L), np.float32), *rargs)[0, 0]
    covered = np.zeros(D, bool)
    rb3s = float(np.asarray(inputs["rho_b3"]).reshape(-1)[0])
    for c, (res, m) in enumerate(zip(results, metas)):
        z = res["y"].reshape(-1).astype(np.float64) + rb3s
        yc = (1.0 / (1.0 + np.exp(-z))).astype(np.float32)
        sl = m["slot"][m["own_lo"]:m["own_hi"]]
        ge = m["e0"] + np.arange(m["own_lo"], m["own_hi"])
        y[ge] = yc[sl]
        covered[ge] = True
    # patch uncovered (boundary) events exactly on host
    missing = np.nonzero(~covered)[0]
    if len(missing):
        starts = np.searchsorted(r, missing, side="left")
        ends = np.searchsorted(r, missing, side="right")
        for e, st, en in zip(missing, starts, ends):
            h = phi_numpy(x[st:en], *args)
            pooled = h.sum(0, keepdims=True)
            y[e] = rho_numpy(pooled, *rargs)[0, 0]
    return y.reshape(-1, 1)


def _numpy_fallback(inputs, n_events):
    """Reference-exact host computation (used only if the input does not fit
    the compiled schedule)."""
    x = np.asarray(inputs["x"], np.float32)
    args = [np.asarray(inputs[k], np.float32) for k in
            ("phi_w1", "phi_b1", "phi_w2", "phi_b2")]
    rargs = [np.asarray(inputs[k], np.float32) for k in
             ("rho_w1", "rho_b1", "rho_w2", "rho_b2", "rho_w3", "rho_b3")]
    h = phi_numpy(x, *args)
    r = compact_ranks(inputs["event_ids"])
    pooled = np.zeros((n_events, h.shape[1]), np.float32)
    np.add.at(pooled, r, h)
    return rho_numpy(pooled, *rargs).astype(np.float32)


def kernel(**inputs):
    cfg = FULL_CFG
    T = np.asarray(inputs["x"]).shape[0]
    n_events = 100_000
    if T != cfg.n_cores * cfg.T_core:
        return _numpy_fallback(inputs, n_events)
    try:
        in_maps, metas, r, D = prepare_in_maps(inputs, cfg)
    except (ScheduleOverflow, AssertionError):
        return _numpy_fallback(inputs, n_events)
    nc = _get_program(cfg)
    global _LAST_RES
    trace = TRACE and _install_ntff_hook()
    res = run_bass_kernel_spmd(nc, in_maps, core_ids=list(range(cfg.n_cores)),
                               trace=trace)
    _LAST_RES = res
    return assemble_output(res.results, metas, r, D, inputs, cfg, n_events)


# revision 57
# speedup vs baseline: 1.1778x; 1.1778x over previous
"""DeepSet (segment_reduce) Trainium2 kernel — v2.

Model (per reference):
    h  = relu(relu(x @ w1 + b1) @ w2 + b2)          # phi, per track
    pooled[e] = sum_{t in event e} h[t]             # segment sum (sorted ids)
    y  = sigmoid(relu(relu(pooled@rw1+rb1)@rw2+rb2)@rw3+rb3)   # rho, per event

Strategy (8 NeuronCores, SPMD single program):
  - Shard tracks in fixed 250k blocks per core (NOT event aligned); the few
    boundary events that straddle cores are recomputed exactly on the host
    (tiny) and patched into the output.
  - Host reorders x into a transposed interleaved layout xt4 (bf16) so the
    device streams it with contiguous DMA straight into the PE.
  - phi mm1: one [64,128]-stationary matmul per (hab, h) with row tiling so
    the two hab halves run concurrently on the PE; ACT applies bias+relu.
  - phi mm2: h1 chunks as stationary (bf16 128-col -> FWL), w2stk moving.
  - pooling (mm3): FLIPPED vs v1 — stationary = h2 chunk [128 trk, 64 lat],
    moving = a host-precomputed onehot [128 trk, 64 slot-cols] (fp8, DMA'd),
    output = pooledT [64 lat, slot-cols] accumulated into PSUM banks that
    hold 512 slot-columns each.  Slots live on PSUM *columns*, so windows
    need no 32-partition alignment; each bank is cleared up front (memset)
    and every matmul uses start=False (DVE writes don't set has_written, so
    the first matmul overwrites the zeros and the rest accumulate).  Only
    windows that cross a 512-col bank boundary (w % 16 == 15) split into
    two matmuls.
  - rho: operates directly on pooledT slices (no PE transposes), bf16.
  - Boundary events / event ids that never appear are patched on host.
"""

import numpy as np
import ml_dtypes
from contextlib import ExitStack

import concourse.bass as bass
import concourse.tile as tile
from concourse import bacc, mybir
from concourse.bass_utils import run_bass_kernel_spmd

BF16 = ml_dtypes.bfloat16
FP8 = ml_dtypes.float8_e4m3
FP32 = np.float32
AF = mybir.ActivationFunctionType
ALU = mybir.AluOpType
dt = mybir.dt


class Cfg:
    def __init__(self, n_cores=8, tracks_per_core=250_000, tiles_per_window=4,
                 oh_dtype="float8e4", clear_engine="vector",
                 h2_act_every=0, flush_engine="vector", rho_dtype="float16"):
        self.n_cores = n_cores
        self.F = 32           # input features
        self.L = 64           # latent width (phi hidden and output width)
        self.RH = 128         # rho hidden width
        self.T_core = tracks_per_core
        self.G = 4096         # tracks per DMA super-tile
        self.TPAD = ((tracks_per_core + 128 + self.G - 1) // self.G) * self.G
        self.NT = self.TPAD // 128          # 128-track tiles per core
        self.TPW = tiles_per_window         # tiles per 32-slot window
        self.NWIN = self.NT // self.TPW     # windows (tile i -> window i//TPW)
        # slot columns: window w covers cols [32w, 32w+64)
        self.SPAD = ((32 * (self.NWIN - 1) + 64 + 511) // 512) * 512
        self.NBANK = self.SPAD // 512       # 512-slot psum banks
        self.oh_dtype = oh_dtype
        self.clear_engine = clear_engine
        self.h2_act_every = h2_act_every    # every k-th h2 relu on ACT (0=never)
        self.flush_engine = flush_engine
        self.rho_dtype = rho_dtype

    def __repr__(self):
        return (f"Cfg2(cores={self.n_cores},TPAD={self.TPAD},NT={self.NT},"
                f"TPW={self.TPW},SPAD={self.SPAD},oh={self.oh_dtype},"
                f"clr={self.clear_engine},h2a={self.h2_act_every},"
                f"fl={self.flush_engine},rho={self.rho_dtype})")


FULL_CFG = Cfg()


# --------------------------------------------------------------------------
# Host-side planning
# --------------------------------------------------------------------------

class ScheduleOverflow(Exception):
    pass


def compact_ranks(event_ids):
    ev = np.asarray(event_ids)
    change = (ev[1:] != ev[:-1]).astype(np.int64)
    r = np.concatenate([[0], np.cumsum(change)]).astype(np.int64)
    return r


def plan_core(r_local, cfg):
    """Assign slot columns to local events.

    r_local: int64 [Tc] local event ranks (0-based, non-decreasing).
    Returns (rel int16 [TPAD] per-track window-relative column (or -1),
             slot int64 [n_local_events] global slot column per event).
    """
    Tc = len(r_local)
    n_ev = int(r_local[-1]) + 1 if Tc else 0
    first_track = np.searchsorted(r_local, np.arange(n_ev), side="left")
    last_track = np.searchsorted(r_local, np.arange(n_ev), side="right") - 1
    first_w = (first_track // 128) // cfg.TPW
    last_w = (last_track // 128) // cfg.TPW

    slot = np.zeros(n_ev, dtype=np.int64)
    counter = 0
    lo = 32 * last_w
    for e in range(n_ev):
        counter = max(counter, lo[e])
        slot[e] = counter
        counter += 1
    rel_hi = slot - 32 * first_w
    if rel_hi.max(initial=0) >= 64:
        raise ScheduleOverflow(f"max rel {rel_hi.max()} >= 64")
    if slot.max(initial=0) >= cfg.SPAD:
        raise ScheduleOverflow("slot overflow")

    tiles = np.arange(cfg.TPAD) // 128
    rel = np.full(cfg.TPAD, -1, dtype=np.int64)
    rel[:Tc] = slot[r_local] - 32 * (tiles[:Tc] // cfg.TPW)
    assert rel[:Tc].min(initial=0) >= 0 and rel[:Tc].max(initial=0) < 64
    return rel, slot


def make_xt4(x_pad, cfg):
    """[TPAD, F] f32 -> [128, TPAD//4] interleaved transposed layout.

    track t = 4096 g + 1024 b + j maps to partition 32 b + f, column
    1024 g + j.
    """
    G = cfg.G
    ng = cfg.TPAD // G
    xt = x_pad.reshape(ng, 4, G // 4, cfg.F).transpose(1, 3, 0, 2)
    return np.ascontiguousarray(xt.reshape(128, -1))


def emission_order(cfg):
    """Tile indices in device processing order (must match build_program)."""
    order = []
    for g in range(cfg.TPAD // cfg.G):
        for hab in range(2):
            i0 = 32 * g + 16 * hab
            for t2 in range(2):
                for m in range(8):
                    order.append(i0 + 8 * (m % 2) + 4 * t2 + m // 2)
    return order


def make_onehot(rel, cfg):
    """[TPAD] window-relative cols -> [128, NT*64] onehot in emission order."""
    order = emission_order(cfg)
    rel_t = rel.reshape(cfg.NT, 128)[order]          # [NT(pos), 128]
    oh = rel_t[:, :, None] == np.arange(64)[None, None, :]
    np_dt = FP8 if cfg.oh_dtype == "float8e4" else BF16
    oh = oh.transpose(1, 0, 2).reshape(128, cfg.NT * 64)
    return np.ascontiguousarray(oh.astype(np_dt))


def phi_numpy(x, w1, b1, w2, b2, *unused):
    h = np.maximum(x @ w1 + b1, 0.0)
    h = np.maximum(h @ w2 + b2, 0.0)
    return h


def rho_numpy(pooled, rw1, rb1, rw2, rb2, rw3, rb3):
    r = np.maximum(pooled @ rw1 + rb1, 0.0)
    r = np.maximum(r @ rw2 + rb2, 0.0)
    z = r @ rw3 + rb3
    return 1.0 / (1.0 + np.exp(-z))


# --------------------------------------------------------------------------
# Device program
# --------------------------------------------------------------------------

def build_program(cfg):
    nc = bacc.Bacc("TRN2", target_bir_lowering=False, debug=False,
                   enable_asserts=False, num_devices=cfg.n_cores)
    F, L, RH = cfg.F, cfg.L, cfg.RH
    NT, TPW = cfg.NT, cfg.TPW
    ohdt = getattr(dt, cfg.oh_dtype)
    rhodt = getattr(dt, cfg.rho_dtype)

    xt4_d = nc.dram_tensor("xt4", [128, cfg.TPAD // 4], dt.bfloat16,
                           kind="ExternalInput").ap()
    oh_d = nc.dram_tensor("ohd", [128, NT * 64], ohdt,
                          kind="ExternalInput").ap()
    w1_d = nc.dram_tensor("w1blk", [128, 256], dt.bfloat16,
                          kind="ExternalInput").ap()
    b1_d = nc.dram_tensor("b1rep", [128, 1], dt.float32,
                          kind="ExternalInput").ap()
    w2_d = nc.dram_tensor("w2stk", [128, 128], dt.bfloat16,
                          kind="ExternalInput").ap()
    rw1_d = nc.dram_tensor("rw1", [64, RH], rhodt,
                           kind="ExternalInput").ap()
    rb1_d = nc.dram_tensor("rb1", [128, 1], dt.float32,
                           kind="ExternalInput").ap()
    rw2_d = nc.dram_tensor("rw2", [128, L], rhodt,
                           kind="ExternalInput").ap()
    rb2_d = nc.dram_tensor("rb2", [64, 1], dt.float32,
                           kind="ExternalInput").ap()
    rw3_d = nc.dram_tensor("rw3", [64, 1], rhodt,
                           kind="ExternalInput").ap()
    y_d = nc.dram_tensor("y", [1, cfg.SPAD], dt.float32,
                         kind="ExternalOutput").ap()

    NG = cfg.TPAD // cfg.G

    with tile.TileContext(nc) as tc, ExitStack() as ctx:
        const = ctx.enter_context(tc.tile_pool(name="const", bufs=1))
        w1_s = const.tile([128, 256], dt.bfloat16, tag="w1")
        b1_s = const.tile([128, 1], dt.float32, tag="b1")
        w2_s = const.tile([128, 128], dt.bfloat16, tag="w2")

        pooled_pool = ctx.enter_context(tc.tile_pool(name="pooled", bufs=1))
        pooled = pooled_pool.tile([64, cfg.SPAD], rhodt)

        clear_eng = getattr(nc, cfg.clear_engine)
        flush_eng = getattr(nc, cfg.flush_engine)

        # rho constants allocated up front: the rho tail is interleaved into
        # the main loop at bank-flush points (DMAs issued after the g=0
        # prefetch below)
        rho_const = ctx.enter_context(tc.tile_pool(name="rhoc", bufs=1))
        rw1_s = rho_const.tile([64, RH], rhodt, tag="rw1")
        rb1_s = rho_const.tile([128, 1], dt.float32, tag="rb1")
        rw2_s = rho_const.tile([128, L], rhodt, tag="rw2")
        rb2_s = rho_const.tile([64, 1], dt.float32, tag="rb2")
        rw3_s = rho_const.tile([64, 1], rhodt, tag="rw3")

        # ---------------- main loop ----------------
        bank_tiles = {}     # bank index -> psum tile object
        with (
            tc.tile_pool(name="xt", bufs=8) as xt_pool,
            tc.tile_pool(name="ohp", bufs=8) as oh_pool,
            tc.tile_pool(name="p1", bufs=2, space="PSUM") as p1_pool,
            tc.tile_pool(name="h1", bufs=8) as h1_pool,
            tc.tile_pool(name="p2", bufs=2, space="PSUM") as p2_pool,
            tc.tile_pool(name="h2", bufs=8) as h2_pool,
            tc.tile_pool(name="p3", bufs=2, space="PSUM") as p3_pool,
            tc.tile_pool(name="r1s", bufs=3) as r1s_pool,
            tc.tile_pool(name="r2s", bufs=3) as r2s_pool,
            tc.tile_pool(name="ys", bufs=2) as ys_pool,
        ):
            def get_bank(B):
                if B not in bank_tiles:
                    bt = p3_pool.tile([64, 512], dt.float32, tag="bank",
                                      name=f"bank{B}")
                    clear_eng.memset(bt[:], 0.0)
                    bank_tiles[B] = bt
                return bank_tiles[B]

            def flush_bank(B):
                bt = bank_tiles.pop(B)
                flush_eng.tensor_copy(pooled[:, 512 * B:512 * (B + 1)], bt[:])

            def mm3(i, h2_ap, oh_ap):
                w = i // TPW
                c = 32 * w
                B, off = c // 512, c % 512
                if off <= 448:
                    nc.tensor.matmul(get_bank(B)[:, off:off + 64],
                                     h2_ap, oh_ap,
                                     start=False, stop=True,
                                     skip_group_check=True)
                else:  # window straddles two banks: split 32 + 32
                    nc.tensor.matmul(get_bank(B)[:, off:off + 32],
                                     h2_ap, oh_ap[:, 0:32],
                                     start=False, stop=True,
                                     skip_group_check=True)
                    nc.tensor.matmul(get_bank(B + 1)[:, 0:32],
                                     h2_ap, oh_ap[:, 32:64],
                                     start=False, stop=True,
                                     skip_group_check=True)

            # rho tail stages, interleaved at bank-flush points; PSUM for the
            # small tail matmuls is borrowed from the p2 pool (all tail tiles
            # are consumed within the same super-tile)
            stages = {}

            def tail_a(B):
                r1p = p2_pool.tile([128, 512], dt.float32, tag="p2",
                                   name="r1p")
                nc.tensor.matmul(r1p[:], rw1_s[:],
                                 pooled[:, 512 * B:512 * (B + 1)],
                                 start=True, stop=True)
                r1s = r1s_pool.tile([128, 512], rhodt, tag="r1s")
                nc.scalar.activation(r1s[:], r1p[:], AF.Relu, bias=rb1_s[:])
                stages[("b", B)] = r1s

            def tail_b(B):
                r1s = stages.pop(("b", B))
                r2p = p2_pool.tile([128, 512], dt.float32, tag="p2",
                                   name="r2p")
                nc.tensor.matmul(r2p[0:64, :], rw2_s[:], r1s[:],
                                 start=True, stop=True)
                r2s = r2s_pool.tile([64, 512], rhodt, tag="r2s")
                nc.scalar.activation(r2s[:], r2p[0:64, :], AF.Relu,
                                     bias=rb2_s[:])
                stages[("c", B)] = r2s

            def tail_c(B):
                r2s = stages.pop(("c", B))
                yp = p2_pool.tile([128, 512], dt.float32, tag="p2",
                                  name="yp")
                nc.tensor.matmul(yp[0:1, :], rw3_s[:], r2s[:],
                                 start=True, stop=True)
                ys = ys_pool.tile([1, 512], dt.float32, tag="ys")
                nc.vector.tensor_copy(ys[:], yp[0:1, :])
                nc.sync.dma_start(y_d[:, 512 * B:512 * (B + 1)], ys[:])

            # first super-tile and w1 go down the DMA queue ahead of the
            # other constants so mm1 of g=0 starts as early as possible
            xt0 = xt_pool.tile([128, 1024], dt.bfloat16, tag="xt", name="xt0")
            nc.sync.dma_start(xt0[:], xt4_d[:, 0:1024])
            nc.sync.dma_start(w1_s[:], w1_d)
            oh0 = oh_pool.tile([128, 2048], ohdt, tag="oh", name="oh0")
            nc.sync.dma_start(oh0[:], oh_d[:, 0:2048])
            nc.sync.dma_start(b1_s[:], b1_d)
            nc.sync.dma_start(w2_s[:], w2_d)
            nc.sync.dma_start(rw1_s[:], rw1_d)
            nc.sync.dma_start(rb1_s[:], rb1_d)
            nc.sync.dma_start(rw2_s[:], rw2_d)
            nc.sync.dma_start(rb2_s[:], rb2_d)
            nc.sync.dma_start(rw3_s[:], rw3_d)

            t2_count = 0
            for g in range(NG):
                if g == 0:
                    xt_t, oh_t = xt0, oh0
                else:
                    xt_t = xt_pool.tile([128, 1024], dt.bfloat16, tag="xt")
                    nc.sync.dma_start(xt_t[:],
                                      xt4_d[:, 1024 * g:1024 * (g + 1)])
                    oh_t = oh_pool.tile([128, 2048], ohdt, tag="oh")
                    nc.sync.dma_start(oh_t[:],
                                      oh_d[:, 2048 * g:2048 * (g + 1)])
                # eagerly clear the pooling bank this super-tile will first
                # touch, so the memset overlaps mm1/mm2 instead of stalling
                # the first pooling matmul
                need = g // 2 if g % 2 == 0 else (g + 1) // 2
                if need < cfg.NBANK:
                    get_bank(need)
                p1s = [p1_pool.tile([128, 1024], dt.float32, tag="p1",
                                    name=f"p1_{hab}") for hab in range(2)]
                # full-row stationaries (zero half per hab): untiled LDWEIGHTS
                # can load into the background buffer and hide behind matmuls
                for h in range(2):
                    for hab in range(2):
                        nc.tensor.matmul(
                            p1s[hab][:, 512 * h:512 * (h + 1)],
                            w1_s[:, 128 * hab:128 * (hab + 1)],
                            xt_t[:, 512 * h:512 * (h + 1)],
                            start=True, stop=True)
                h1s = []
                for hab in range(2):
                    h1 = h1_pool.tile([128, 1024], dt.bfloat16, tag="h1",
                                      name=f"h1_{hab}")
                    nc.scalar.activation(h1[:], p1s[hab][:],
                                         AF.Relu, bias=b1_s[:])
                    h1s.append(h1)
                if g % 2 == 0 and g >= 2:
                    # oldest stage first: every tail matmul's input is >=1
                    # super-tile old, so the PE never waits here
                    A = (g - 2) // 2
                    if A >= 2:
                        tail_c(A - 2)
                    if A >= 1:
                        tail_b(A - 1)
                    tail_a(A)
                for hab in range(2):
                    h1 = h1s[hab]
                    i0 = 32 * g + 16 * hab
                    for t2 in range(2):
                        p2 = p2_pool.tile([128, 512], dt.float32, tag="p2")
                        for m4 in range(4):
                            j = 4 * t2 + m4
                            nc.tensor.matmul(
                                p2[:, 128 * m4:128 * (m4 + 1)],
                                h1[:, 128 * j:128 * (j + 1)],
                                w2_s[:],
                                start=True, stop=True)
                        h2 = h2_pool.tile([128, 512], dt.bfloat16, tag="h2")
                        t2_count += 1
                        if (cfg.h2_act_every and
                                t2_count % cfg.h2_act_every == 0):
                            nc.scalar.activation(h2[:], p2[:], AF.Relu)
                        else:
                            nc.vector.tensor_scalar_max(h2[:], p2[:], 0.0)
                        ohbase = 64 * (16 * hab + 8 * t2)
                        for m in range(8):
                            i = i0 + 8 * (m % 2) + 4 * t2 + m // 2
                            mm3(i, h2[:, 64 * m:64 * (m + 1)],
                                oh_t[:, ohbase + 64 * m:ohbase + 64 * (m + 1)])
                if g % 2 == 1:
                    B = (g - 1) // 2
                    if B in bank_tiles:
                        flush_bank(B)
            for B in sorted(bank_tiles):
                flush_bank(B)
            # drain the remaining tail stages
            NBK = cfg.NBANK
            last_even = NG - 1 - ((NG - 1) % 2)
            a_next = (last_even - 2) // 2 + 1 if last_even >= 2 else 0
            for A in range(a_next, NBK + 3):
                if 0 <= A - 2 < NBK:
                    tail_c(A - 2)
                if 1 <= A - 1 < NBK:
                    tail_b(A - 1)
                if A < NBK:
                    tail_a(A)

    nc.compile()
    return nc


# --------------------------------------------------------------------------
# kernel() entry point
# --------------------------------------------------------------------------

_PROG_CACHE = {}
TRACE = False
_LAST_RES = None


def _install_ntff_hook():
    """Register the axon NTFF profiling hook if the image lacks
    antenv.axon_hooks (needed for run_bass_kernel_spmd(trace=True))."""
    import sys, types
    try:
        from antenv.axon_hooks import get_axon_ntff_profile_hook  # noqa: F401
        return True
    except ImportError:
        pass
    try:
        from trn_agent_boot.trn_boot import _ntff_profile_via_ctypes
        hook = _ntff_profile_via_ctypes("/opt/axon/libaxon_pjrt.so")
        if hook is None:
            return False
        mod = types.ModuleType("antenv.axon_hooks")
        mod.get_axon_ntff_profile_hook = lambda: hook
        mod.set_axon_ntff_profile_hook = lambda h: None
        sys.modules["antenv.axon_hooks"] = mod
        return True
    except Exception:
        return False


def _get_program(cfg):
    key = repr(cfg)
    if key not in _PROG_CACHE:
        _PROG_CACHE[key] = build_program(cfg)
    return _PROG_CACHE[key]


def prepare_in_maps(inputs, cfg):
    x = np.asarray(inputs["x"], np.float32)
    ev = np.asarray(inputs["event_ids"])
    w1 = np.asarray(inputs["phi_w1"], np.float32)
    b1 = np.asarray(inputs["phi_b1"], np.float32)
    w2 = np.asarray(inputs["phi_w2"], np.float32)
    b2 = np.asarray(inputs["phi_b2"], np.float32)
    assert np.all(b2 == 0.0), "phi_b2 != 0 unsupported fast path"
    assert np.all(np.asarray(inputs["rho_b2"]) == 0.0), \
        "rho_b2 != 0 unsupported fast path"
    T = x.shape[0]
    r = compact_ranks(ev)
    D = int(r[-1]) + 1

    rho_np = {"bfloat16": BF16, "float16": np.float16}.get(
        cfg.rho_dtype, np.float32)

    blk = np.zeros((64, 128), np.float32)
    blk[0:32, 0:64] = w1
    blk[32:64, 64:128] = w1
    z = np.zeros((64, 128), np.float32)
    w1blk = np.hstack([np.vstack([blk, z]),
                       np.vstack([z, blk])]).astype(BF16)
    w2stk = np.zeros((128, 128), np.float32)
    w2stk[0:64, 0:64] = w2
    w2stk[64:128, 64:128] = w2
    w2stk = w2stk.astype(BF16)
    b1rep = np.tile(b1.reshape(-1), 2).reshape(128, 1).astype(np.float32)
    rw1 = np.asarray(inputs["rho_w1"], np.float32).astype(rho_np)
    rb1 = np.asarray(inputs["rho_b1"], np.float32).reshape(128, 1)
    rw2 = np.asarray(inputs["rho_w2"], np.float32).astype(rho_np)
    rb2 = np.asarray(inputs["rho_b2"], np.float32).reshape(64, 1)
    rw3 = np.asarray(inputs["rho_w3"], np.float32).astype(rho_np)

    in_maps, metas = [], []
    for c in range(cfg.n_cores):
        s, e = c * cfg.T_core, min((c + 1) * cfg.T_core, T)
        r_loc_g = r[s:e]
        e0 = int(r_loc_g[0])
        r_loc = (r_loc_g - e0).astype(np.int64)
        rel, slot = plan_core(r_loc, cfg)
        xp = np.zeros((cfg.TPAD, cfg.F), np.float32)
        xp[:e - s] = x[s:e]
        in_maps.append({
            "xt4": make_xt4(xp, cfg).astype(BF16),
            "ohd": make_onehot(rel, cfg),
            "w1blk": w1blk, "b1rep": b1rep, "w2stk": w2stk,
            "rw1": rw1, "rb1": rb1, "rw2": rw2, "rb2": rb2, "rw3": rw3,
        })
        # events fully owned by this core (not straddling boundary)
        n_ev = int(r_loc[-1]) + 1
        own_lo = 0 if s == 0 else (1 if r[s - 1] == r[s] else 0)
        own_hi = n_ev if e == T else (n_ev - 1 if r[e - 1] == r[e] else n_ev)
        metas.append(dict(e0=e0, n_ev=n_ev, own_lo=own_lo, own_hi=own_hi,
                          slot=slot))
    return in_maps, metas, r, D


def assemble_output(results, metas, r, D, inputs, cfg, n_events):
    x = np.asarray(inputs["x"], np.float32)
    args = [np.asarray(inputs[k], np.float32) for k in
            ("phi_w1", "phi_b1", "phi_w2", "phi_b2")]
    rargs = [np.asarray(inputs[k], np.float32) for k in
             ("rho_w1", "rho_b1", "rho_w2", "rho_b2", "rho_w3", "rho_b3")]
    y = np.empty(n_events, np.float32)
    if D < n_events:
        y[D:] = rho_numpy(np.zeros((1, cfg.L), np.float32), *rargs)[0, 0]
    covered = np.zeros(D, bool)
    rb3s = float(np.asarray(inputs["rho_b3"]).reshape(-1)[0])
    for c, (res, m) in enumerate(zip(results, metas)):
        z = res["y"].reshape(-1).astype(np.float64) + rb3s
        yc = (1.0 / (1.0 + np.exp(-z))).astype(np.float32)
        sl = m["slot"][m["own_lo"]:m["own_hi"]]
        ge = m["e0"] + np.arange(m["own_lo"], m["own_hi"])
        y[ge] = yc[sl]
        covered[ge] = True
    # patch uncovered (boundary) events exactly on host
    missing = np.nonzero(~covered)[0]
    if len(missing):
        starts = np.searchsorted(r, missing, side="left")
        ends = np.searchsorted(r, missing, side="right")
        for e, st, en in zip(missing, starts, ends):
            h = phi_numpy(x[st:en], *args)
            pooled = h.sum(0, keepdims=True)
            y[e] = rho_numpy(pooled, *rargs)[0, 0]
    return y.reshape(-1, 1)


def _numpy_fallback(inputs, n_events):
    """Reference-exact host computation (used only if the input does not fit
    the compiled schedule)."""
    x = np.asarray(inputs["x"], np.float32)
    args = [np.asarray(inputs[k], np.float32) for k in
            ("phi_w1", "phi_b1", "phi_w2", "phi_b2")]
    rargs = [np.asarray(inputs[k], np.float32) for k in
             ("rho_w1", "rho_b1", "rho_w2", "rho_b2", "rho_w3", "rho_b3")]
    h = phi_numpy(x, *args)
    r = compact_ranks(inputs["event_ids"])
    pooled = np.zeros((n_events, h.shape[1]), np.float32)
    np.add.at(pooled, r, h)
    return rho_numpy(pooled, *rargs).astype(np.float32)


def kernel(**inputs):
    cfg = FULL_CFG
    T = np.asarray(inputs["x"]).shape[0]
    n_events = 100_000
    if T != cfg.n_cores * cfg.T_core:
        return _numpy_fallback(inputs, n_events)
    try:
        in_maps, metas, r, D = prepare_in_maps(inputs, cfg)
    except (ScheduleOverflow, AssertionError):
        return _numpy_fallback(inputs, n_events)
    nc = _get_program(cfg)
    global _LAST_RES
    trace = TRACE and _install_ntff_hook()
    res = run_bass_kernel_spmd(nc, in_maps, core_ids=list(range(cfg.n_cores)),
                               trace=trace)
    _LAST_RES = res
    return assemble_output(res.results, metas, r, D, inputs, cfg, n_events)
